# revision 12
# baseline (speedup 1.0000x reference)
"""Trainium2 Bass kernel for nn_DegreeEmbeddingNetwork (gnn_message_passing).

Strategy (8 NeuronCores, SPMD single program):
  - The reference collapses: node features are a constant broadcast
    (s0 = lin_w + lin_b) and the l=1 node block is structurally zero, so
        h   = scalars @ rad_w1                  (radial MLP layer 1)
        h2  = silu(LN(h))                       (per-edge layernorm over 64)
        q   = h2 @ B  (+ c)                     (B folds rad_w2 x TP x proj)
        deg = [a0*q0 | outer(q1, a1)]           (160 wide)
        out = scatter_add(deg by dst) / sqrt(32)
  - Host folds: LN mean into W1c (linear), LN gain g into W1 columns, and
    the per-edge rstd into the edge features themselves (x' = x * rstd —
    row scaling commutes through the right-matmul).  beta enters as the
    activation bias (per-partition in the hid-major layout).  rad_off /
    proj_b0 are exact rank-1 host-side corrections.
  - Device per 4-tile batch (128 edges/tile), all matmuls bf16:
      MM1T (lhsT=W1g, rhs=xT chunk)      -> H2T-pre [64, 512] PSUM
      ACT silu(+beta bias)               -> H2T [64,512] SBUF bf16
      MM2 x4 (lhsT=H2T tile, rhs=B)      -> Q [128, 4x96] PSUM
      deg build (a0*q0, q1 x a1)         -> deg bf16 (DVE/Pool split)
      onehot (iota == idx)               -> oh bf16 (DVE/Pool split)
      scatter matmul (lhsT=oh, rhs=deg)  -> window accumulator in PSUM
  - Edges sorted by destination node; core k owns nodes [k*NPC,(k+1)*NPC);
    no collectives, host concatenates the 8 node shards.
"""

import math
import sys

sys.path.insert(0, "/opt/trn_rl_repo")

import numpy as np

try:
    import ml_dtypes
    BF16_NP = ml_dtypes.bfloat16
except Exception:  # pragma: no cover
    BF16_NP = None

import concourse.bacc as bacc
import concourse.tile as tile
from concourse import mybir
from concourse.bass_utils import run_bass_kernel_spmd

F32 = mybir.dt.float32
BF16 = mybir.dt.bfloat16

N_CORES = 8
MUL0, MUL1 = 64, 32
D_EMB = 160
RAD_HID = 64
AVG_AGG = 32.0
LN_EPS = 1e-5
WIN = 128          # nodes per scatter window
SUP = 4            # tiles per batch (one PSUM bank of Q)
SGT = 16           # tiles per DMA chunk
DEG_W = 160

CONFIG = {
    "deg1_engine": "dve",     # "dve" | "pool" | "mix"
    "deg1_dve_every": 4,       # if "mix": 1 of every N batches on DVE
    "deg0_engine": "dve",      # "dve" | "pool" | "mix"
    "deg0_pool_every": 4,      # if "mix": 1 of every N batches on Pool
    "onehot_engine": "dve",    # "dve" | "pool" | "mix"
    "onehot_pool_every": 2,    # if "mix": 1 of every N tiles on Pool
}

_PROGRAM_CACHE = {}
_LAST_IN_MAPS = None


def build_program(C, NW, TPW, NT):
    """SPMD Bass program. C = padded edges/core, NW windows of 128 nodes,
    TPW tiles per window, NT = NW*TPW total tiles (multiple of SGT)."""
    nc = bacc.Bacc("TRN2", target_bir_lowering=False, debug=False,
                   num_devices=N_CORES)

    xt_d = nc.dram_tensor("xt", [64, C], BF16, kind="ExternalInput").ap()
    aux_d = nc.dram_tensor("aux", [NT // SGT, 128, SGT * 5], F32,
                           kind="ExternalInput").ap()
    w1_d = nc.dram_tensor("w1g", [64, 64], BF16, kind="ExternalInput").ap()
    b_d = nc.dram_tensor("bmat", [64, 96], BF16, kind="ExternalInput").ap()
    beta_d = nc.dram_tensor("betac", [64, 1], F32, kind="ExternalInput").ap()
    iota_d = nc.dram_tensor("iota", [128, WIN], BF16, kind="ExternalInput").ap()
    out_d = nc.dram_tensor("out", [NW * 128, D_EMB], F32,
                           kind="ExternalOutput").ap()

    assert NT % SGT == 0 and SGT % SUP == 0

    with tile.TileContext(nc) as tc:
        with (
            tc.tile_pool(name="consts", bufs=1) as cpool,
            tc.tile_pool(name="xt", bufs=3) as xt_pool,
            tc.tile_pool(name="aux", bufs=3) as aux_pool,
            tc.tile_pool(name="h2t", bufs=4) as h2t_pool,
            tc.tile_pool(name="oh", bufs=8) as oh_pool,
            tc.tile_pool(name="deg", bufs=1) as deg_pool,
            tc.tile_pool(name="flush", bufs=3) as fl_pool,
            tc.tile_pool(name="psH", bufs=3, space="PSUM") as psH,
            tc.tile_pool(name="psQT", bufs=3, space="PSUM") as psQT,
            tc.tile_pool(name="psA", bufs=2, space="PSUM") as psA,
        ):
            w1_sb = cpool.tile([64, 64], BF16)
            nc.sync.dma_start(w1_sb[:], w1_d[:])
            b_sb = cpool.tile([64, 96], BF16)
            nc.sync.dma_start(b_sb[:], b_d[:])
            beta_sb = cpool.tile([64, 1], F32)
            nc.sync.dma_start(beta_sb[:], beta_d[:])
            iota_sb = cpool.tile([128, WIN], BF16)
            nc.sync.dma_start(iota_sb[:], iota_d[:])

            # fixed deg buffers (bf16), zeroed once
            NDEG = 4
            deg_bufs = []
            for i in range(NDEG):
                d = deg_pool.tile([128, SUP * DEG_W], BF16, tag=f"deg{i}")
                nc.vector.memset(d[:].bitcast(F32), 0.0)
                deg_bufs.append(d)

            def winof(nt):
                return min(nt // TPW, NW - 1)

            acc = None
            acc_win = -1

            for sg in range(NT // SGT):
                xtg = xt_pool.tile([64, SGT * 128], BF16)
                nc.sync.dma_start(
                    xtg[:], xt_d[:, sg * SGT * 128:(sg + 1) * SGT * 128])
                auxg = aux_pool.tile([128, SGT * 5], F32)
                nc.sync.dma_start(auxg[:], aux_d[sg])

                for si in range(SGT // SUP):
                    nt0 = sg * SGT + si * SUP
                    # ---- MM1T: one matmul, 512 moving cols ----
                    psHn = psH.tile([64, SUP * 128], F32)
                    nc.tensor.matmul(
                        psHn[:], w1_sb[:],
                        xtg[:, si * SUP * 128:(si + 1) * SUP * 128],
                        start=True, stop=True)
                    # ---- silu (+beta) hid-major, PSUM -> SBUF bf16 ----
                    H2T = h2t_pool.tile([64, SUP * 128], BF16)
                    nc.scalar.activation(
                        H2T[:], psHn[:],
                        mybir.ActivationFunctionType.Silu,
                        bias=beta_sb[:], scale=1.0)
                    # ---- MM2 per tile ----
                    QT = psQT.tile([128, SUP * 96], F32)
                    for t in range(SUP):
                        nc.tensor.matmul(
                            QT[:, t * 96:(t + 1) * 96],
                            H2T[:, t * 128:(t + 1) * 128],
                            b_sb[:],
                            start=True, stop=True)

                    a3 = (auxg[:, si * SUP * 5:(si + 1) * SUP * 5]
                          .rearrange("p (t f) -> p t f", f=5))
                    deg4 = deg_bufs[(nt0 // SUP) % NDEG]
                    d3 = deg4[:].rearrange("p (t f) -> p t f", f=DEG_W)
                    q3 = QT[:].rearrange("p (t f) -> p t f", f=96)

                    bi = nt0 // SUP
                    # deg0 = a0 * q0  (batched; engine per config)
                    deg0_eng = CONFIG["deg0_engine"]
                    if deg0_eng == "mix":
                        deg0_eng = ("pool" if bi
                                    % CONFIG["deg0_pool_every"] == 0 else "dve")
                    d0_eng = nc.gpsimd if deg0_eng == "pool" else nc.vector
                    a0ex = a3[:, :, 0:1].broadcast_to([128, SUP, 64])
                    d0_eng.scalar_tensor_tensor(
                        d3[:, :, 0:64], q3[:, :, 0:64], 0.0, a0ex,
                        mybir.AluOpType.bypass, mybir.AluOpType.mult)

                    # deg1[m-major] = q1 x a1  (engine per config)
                    deg1_eng = CONFIG["deg1_engine"]
                    if deg1_eng == "mix":
                        deg1_eng = ("dve" if bi
                                    % CONFIG["deg1_dve_every"] == 0 else "pool")
                    if deg1_eng == "pool":
                        dv = d3[:, :, 64:160].rearrange(
                            "p t (m v) -> p t m v", m=3)
                        q1ex = (q3[:, :, 64:96].unsqueeze(2)
                                .broadcast_to([128, SUP, 3, 32]))
                        a1ex = (a3[:, :, 1:4].unsqueeze(3)
                                .broadcast_to([128, SUP, 3, 32]))
                        nc.gpsimd.scalar_tensor_tensor(
                            dv, q1ex, 0.0, a1ex,
                            mybir.AluOpType.bypass, mybir.AluOpType.mult)
                    else:
                        for m_ in range(3):
                            a1ex = (a3[:, :, 1 + m_:2 + m_]
                                    .broadcast_to([128, SUP, 32]))
                            nc.vector.scalar_tensor_tensor(
                                d3[:, :, 64 + 32 * m_:96 + 32 * m_],
                                q3[:, :, 64:96], 0.0, a1ex,
                                mybir.AluOpType.bypass, mybir.AluOpType.mult)

                    # ---- onehot + scatter per tile ----
                    for t in range(SUP):
                        nt = nt0 + t
                        w = winof(nt)
                        tin = nt - w * TPW
                        oh_eng_name = CONFIG["onehot_engine"]
                        if oh_eng_name == "mix":
                            oh_eng_name = ("pool" if nt
                                           % CONFIG["onehot_pool_every"] == 0
                                           else "dve")
                        oh_eng = nc.gpsimd if oh_eng_name == "pool" else nc.vector
                        oh = oh_pool.tile([128, WIN], BF16)
                        oh_eng.tensor_scalar(oh[:], iota_sb[:],
                                             a3[:, t, 4:5], None,
                                             mybir.AluOpType.is_equal)
                        if tin == 0:
                            if acc is not None:
                                fl = fl_pool.tile([128, D_EMB], F32)
                                nc.vector.tensor_copy(fl[:], acc[:, 0:D_EMB])
                                nc.sync.dma_start(
                                    out_d[acc_win * 128:(acc_win + 1) * 128, :],
                                    fl[:])
                            acc = psA.tile([128, DEG_W], F32)
                            acc_win = w
                        is_last = (nt == NT - 1) or (winof(nt + 1) != w)
                        nc.tensor.matmul(
                            acc[:],
                            oh[:],
                            deg4[:, t * DEG_W:(t + 1) * DEG_W],
                            start=(tin == 0), stop=is_last,
                            skip_group_check=True)

            fl = fl_pool.tile([128, D_EMB], F32)
            nc.vector.tensor_copy(fl[:], acc[:, 0:D_EMB])
            nc.sync.dma_start(out_d[acc_win * 128:(acc_win + 1) * 128, :], fl[:])

    nc.finalize()
    return nc


def kernel(dst_input, src_attr, scalars, lin_w, lin_b, rad_w1, rad_g, rad_beta,
           rad_w2, rad_off, proj_w0, proj_b0, proj_w1, dst_index):
    dst_input = np.asarray(dst_input)
    src_attr = np.asarray(src_attr, np.float32)
    scalars = np.asarray(scalars, np.float32)
    lin_w = np.asarray(lin_w, np.float64)
    lin_b = np.asarray(lin_b, np.float64)
    rad_w1 = np.asarray(rad_w1, np.float32)
    rad_g = np.asarray(rad_g, np.float32)
    rad_beta = np.asarray(rad_beta, np.float32)
    rad_w2 = np.asarray(rad_w2, np.float64)
    rad_off = np.asarray(rad_off, np.float64)
    proj_w0 = np.asarray(proj_w0, np.float64)
    proj_b0 = np.asarray(proj_b0, np.float64)
    proj_w1 = np.asarray(proj_w1, np.float64)
    dst_index = np.asarray(dst_index)

    N = dst_input.shape[0]
    E = scalars.shape[0]
    out_dtype = dst_input.dtype

    # ---- host weight folds ----
    s0 = lin_w + lin_b                                   # [64]
    k0 = 1.0 / (math.sqrt(MUL0 + MUL1) * math.sqrt(AVG_AGG))
    k1 = 1.0 / (math.sqrt(MUL0 + 2 * MUL1) * math.sqrt(AVG_AGG))
    A0 = s0[:, None] * proj_w0[:MUL0, :]                 # [64, 64]
    A1 = s0[:, None] * proj_w1[:MUL0, :]                 # [64, 32]
    B0 = rad_w2[:, 0:64] @ A0 * k0                       # [64, 64]
    B1 = rad_w2[:, 64:128] @ A1 * k1                     # [64, 32]
    Bm = np.concatenate([B0, B1], axis=1)                # [64, 96] (m-major out)
    c0 = rad_off[0:64] @ A0 * k0                         # [64]
    c1 = rad_off[64:128] @ A1 * k1                       # [32]
    W1c = rad_w1 - rad_w1.mean(axis=1, keepdims=True)    # fold LN mean
    W1g = W1c * rad_g[None, :]                           # fold LN gain

    # ---- host rstd fold: x' = x * rstd(x) ----
    hc = scalars @ W1c                                   # [E, 64] f32
    var = np.mean(hc * hc, axis=1) + LN_EPS
    rstd = (1.0 / np.sqrt(var)).astype(np.float32)
    xprime = scalars * rstd[:, None]                     # [E, 64]

    # ---- node -> window bin packing (balance edges per 128-node window) ----
    import heapq
    NPC = (N + N_CORES - 1) // N_CORES
    NW = (NPC + WIN - 1) // WIN                          # windows per core
    NWIN = NW * N_CORES                                  # global window count
    deg = np.bincount(dst_index, minlength=N)
    node_order = np.argsort(-deg, kind="stable")
    heap = [(0, 0, w) for w in range(NWIN)]              # (load, count, win)
    heapq.heapify(heap)
    win_nodes = [[] for _ in range(NWIN)]
    win_load = np.zeros(NWIN, np.int64)
    for nd in node_order:
        while True:
            load, cntn, w = heapq.heappop(heap)
            if cntn < WIN:
                break                                    # full windows drop out
        win_nodes[w].append(int(nd))
        win_load[w] = load + deg[nd]
        heapq.heappush(heap, (int(win_load[w]), cntn + 1, w))
    # windows -> cores: sort by load desc, greedily to least-loaded core
    worder = np.argsort(-win_load, kind="stable")
    core_wins = [[] for _ in range(N_CORES)]
    core_load = np.zeros(N_CORES, np.int64)
    for w in worder:
        k = min((kk for kk in range(N_CORES) if len(core_wins[kk]) < NW),
                key=lambda kk: core_load[kk])
        core_wins[k].append(int(w))
        core_load[k] += win_load[w]

    TPW = max(1, int(np.ceil(win_load.max() / 128)))
    NT = NW * TPW
    NT = ((NT + SGT - 1) // SGT) * SGT                   # pad to DMA chunk
    C = NT * 128

    # per-node edge ranges in the dst-sorted order
    order = np.argsort(dst_index, kind="stable")
    starts = np.zeros(N + 1, np.int64)
    np.cumsum(deg, out=starts[1:])

    key = (C, NW, TPW, NT, tuple(sorted(CONFIG.items())))
    if key not in _PROGRAM_CACHE:
        _PROGRAM_CACHE[key] = build_program(C, NW, TPW, NT)
    nc = _PROGRAM_CACHE[key]

    # ---- per-core input arrays ----
    iota = np.broadcast_to(np.arange(WIN, dtype=np.float32)[None, :],
                           (128, WIN)).astype(BF16_NP)
    w1_b = np.ascontiguousarray(W1g.astype(np.float32)).astype(BF16_NP)
    b_b = np.ascontiguousarray(Bm.astype(np.float32)).astype(BF16_NP)
    beta_c = np.ascontiguousarray(rad_beta.reshape(64, 1).astype(np.float32))

    node_core = np.zeros(N, np.int32)
    node_row = np.zeros(N, np.int64)
    in_maps = []
    for k in range(N_CORES):
        xt = np.zeros((C, 64), np.float32)
        aux = np.zeros((NT, 128, 5), np.float32)
        aux[:, :, 4] = -1.0
        a = aux.reshape(NT * 128, 5)
        for wi, w in enumerate(core_wins[k]):
            nds = np.asarray(win_nodes[w], np.int64)
            if nds.size == 0:
                continue
            node_core[nds] = k
            node_row[nds] = wi * 128 + np.arange(nds.size)
            eidx = np.concatenate(
                [order[starts[nd]:starts[nd + 1]] for nd in nds])
            cnt = eidx.size
            base = wi * TPW * 128
            xt[base:base + cnt] = xprime[eidx]
            a[base:base + cnt, 0] = src_attr[eidx, 0]
            a[base:base + cnt, 1:4] = src_attr[eidx, 1:4]
            a[base:base + cnt, 4] = np.repeat(
                np.arange(nds.size, dtype=np.float32), deg[nds])
        auxp = np.ascontiguousarray(
            aux.reshape(NT // SGT, SGT, 128, 5).transpose(0, 2, 1, 3)
            .reshape(NT // SGT, 128, SGT * 5))
        m = {
            "xt": np.ascontiguousarray(xt.T).astype(BF16_NP),
            "aux": auxp,
            "w1g": w1_b,
            "bmat": b_b,
            "betac": beta_c,
            "iota": iota,
        }
        in_maps.append(m)

    global _LAST_IN_MAPS
    _LAST_IN_MAPS = in_maps
    res = run_bass_kernel_spmd(nc, in_maps, core_ids=list(range(N_CORES)))

    # ---- host assembly ----
    out = np.zeros((N, D_EMB), np.float64)
    for k in range(N_CORES):
        rows = np.asarray(res.results[k]["out"], np.float64)  # [NW*128, 160]
        mask = node_core == k
        out[mask] = rows[node_row[mask]]
    # device o1 layout is m-major (64 + 32*m + v); reference is 64 + 3*v + m
    blk = out[:, 64:160].reshape(N, 3, 32)
    out[:, 64:160] = blk.transpose(0, 2, 1).reshape(N, 96)

    # host-side exact corrections (rad_off and proj_b0 terms)
    if np.any(proj_b0 != 0) or np.any(c0 != 0) or np.any(c1 != 0):
        cnt = np.bincount(dst_index, minlength=N).astype(np.float64)
        suma0 = np.bincount(dst_index, weights=src_attr[:, 0].astype(np.float64),
                            minlength=N)
        out[:, 0:64] += cnt[:, None] * (proj_b0 / math.sqrt(AVG_AGG))[None, :]
        out[:, 0:64] += suma0[:, None] * c0[None, :]
        for m_ in range(3):
            sa = np.bincount(dst_index,
                             weights=src_attr[:, 1 + m_].astype(np.float64),
                             minlength=N)
            out[:, 64 + 3 * np.arange(32) + m_] += sa[:, None] * c1[None, :]

    return out.astype(out_dtype)


# revision 14
# speedup vs baseline: 1.4892x; 1.4892x over previous
"""Trainium2 Bass kernel for nn_DegreeEmbeddingNetwork (gnn_message_passing).

Strategy (8 NeuronCores, SPMD single program):
  - The reference collapses: node features are a constant broadcast
    (s0 = lin_w + lin_b) and the l=1 node block is structurally zero, so
        h   = scalars @ rad_w1                  (radial MLP layer 1)
        h2  = silu(LN(h))                       (per-edge layernorm over 64)
        q   = h2 @ B  (+ c)                     (B folds rad_w2 x TP x proj)
        deg = [a0*q0 | outer(q1, a1)]           (160 wide)
        out = scatter_add(deg by dst) / sqrt(32)
  - Host folds: LN mean into W1c (linear), LN gain g into W1 columns, and
    the per-edge rstd into the edge features themselves (x' = x * rstd —
    row scaling commutes through the right-matmul).  beta enters as the
    activation bias (per-partition in the hid-major layout).  rad_off /
    proj_b0 are exact rank-1 host-side corrections.
  - Device per 4-tile batch (128 edges/tile), all matmuls bf16:
      MM1T (lhsT=W1g, rhs=xT chunk)      -> H2T-pre [64, 512] PSUM
      ACT silu(+beta bias)               -> H2T [64,512] SBUF bf16
      MM2 x4 (lhsT=H2T tile, rhs=B)      -> Q [128, 4x96] PSUM
      deg build (a0*q0, q1 x a1)         -> deg bf16 (DVE/Pool split)
      onehot (iota == idx)               -> oh bf16 (DVE/Pool split)
      scatter matmul (lhsT=oh, rhs=deg)  -> window accumulator in PSUM
  - Edges sorted by destination node; core k owns nodes [k*NPC,(k+1)*NPC);
    no collectives, host concatenates the 8 node shards.
"""

import math
import sys

sys.path.insert(0, "/opt/trn_rl_repo")

import numpy as np

try:
    import ml_dtypes
    BF16_NP = ml_dtypes.bfloat16
except Exception:  # pragma: no cover
    BF16_NP = None

import concourse.bacc as bacc
import concourse.tile as tile
from concourse import mybir
from concourse.bass_utils import run_bass_kernel_spmd

F32 = mybir.dt.float32
BF16 = mybir.dt.bfloat16

N_CORES = 8
MUL0, MUL1 = 64, 32
D_EMB = 160
RAD_HID = 64
AVG_AGG = 32.0
LN_EPS = 1e-5
WIN = 128          # nodes per scatter window
SUP = 4            # tiles per batch (one PSUM bank of Q)
SGT = 16           # tiles per DMA chunk
DEG_W = 160

CONFIG = {
    "deg1_engine": "pool",     # "dve" | "pool" | "mix"
    "deg1_dve_every": 4,       # if "mix": 1 of every N batches on DVE
    "deg0_engine": "dve",      # "dve" | "pool" | "mix"
    "deg0_pool_every": 4,      # if "mix": 1 of every N batches on Pool
    "onehot_engine": "mix",    # "dve" | "pool" | "mix"
    "onehot_pool_every": 16,    # if "mix": 1 of every N tiles on Pool
}

_PROGRAM_CACHE = {}
_LAST_IN_MAPS = None


def build_program(C, NW, TPW, NT):
    """SPMD Bass program. C = padded edges/core, NW windows of 128 nodes,
    TPW tiles per window, NT = NW*TPW total tiles (multiple of SGT)."""
    nc = bacc.Bacc("TRN2", target_bir_lowering=False, debug=False,
                   num_devices=N_CORES)

    xt_d = nc.dram_tensor("xt", [64, C], BF16, kind="ExternalInput").ap()
    aux_d = nc.dram_tensor("aux", [NT // SGT, 128, SGT * 5], F32,
                           kind="ExternalInput").ap()
    w1_d = nc.dram_tensor("w1g", [64, 64], BF16, kind="ExternalInput").ap()
    b_d = nc.dram_tensor("bmat", [64, 96], BF16, kind="ExternalInput").ap()
    beta_d = nc.dram_tensor("betac", [64, 1], F32, kind="ExternalInput").ap()
    iota_d = nc.dram_tensor("iota", [128, WIN], BF16, kind="ExternalInput").ap()
    out_d = nc.dram_tensor("out", [NW * 128, D_EMB], F32,
                           kind="ExternalOutput").ap()

    assert NT % SGT == 0 and SGT % SUP == 0

    with tile.TileContext(nc) as tc:
        with (
            tc.tile_pool(name="consts", bufs=1) as cpool,
            tc.tile_pool(name="xt", bufs=3) as xt_pool,
            tc.tile_pool(name="aux", bufs=3) as aux_pool,
            tc.tile_pool(name="h2t", bufs=4) as h2t_pool,
            tc.tile_pool(name="oh", bufs=8) as oh_pool,
            tc.tile_pool(name="deg", bufs=1) as deg_pool,
            tc.tile_pool(name="flush", bufs=3) as fl_pool,
            tc.tile_pool(name="psH", bufs=3, space="PSUM") as psH,
            tc.tile_pool(name="psQT", bufs=3, space="PSUM") as psQT,
            tc.tile_pool(name="psA", bufs=2, space="PSUM") as psA,
        ):
            w1_sb = cpool.tile([64, 64], BF16)
            nc.sync.dma_start(w1_sb[:], w1_d[:])
            b_sb = cpool.tile([64, 96], BF16)
            nc.sync.dma_start(b_sb[:], b_d[:])
            beta_sb = cpool.tile([64, 1], F32)
            nc.sync.dma_start(beta_sb[:], beta_d[:])
            iota_sb = cpool.tile([128, WIN], BF16)
            nc.sync.dma_start(iota_sb[:], iota_d[:])

            # fixed deg buffers (bf16), zeroed once
            NDEG = 4
            deg_bufs = []
            for i in range(NDEG):
                d = deg_pool.tile([128, SUP * DEG_W], BF16, tag=f"deg{i}")
                nc.vector.memset(d[:].bitcast(F32), 0.0)
                deg_bufs.append(d)

            def winof(nt):
                return min(nt // TPW, NW - 1)

            acc = None
            acc_win = -1

            for sg in range(NT // SGT):
                xtg = xt_pool.tile([64, SGT * 128], BF16)
                nc.sync.dma_start(
                    xtg[:], xt_d[:, sg * SGT * 128:(sg + 1) * SGT * 128])
                auxg = aux_pool.tile([128, SGT * 5], F32)
                nc.sync.dma_start(auxg[:], aux_d[sg])

                for si in range(SGT // SUP):
                    nt0 = sg * SGT + si * SUP
                    # ---- MM1T: one matmul, 512 moving cols ----
                    psHn = psH.tile([64, SUP * 128], F32)
                    nc.tensor.matmul(
                        psHn[:], w1_sb[:],
                        xtg[:, si * SUP * 128:(si + 1) * SUP * 128],
                        start=True, stop=True)
                    # ---- silu (+beta) hid-major, PSUM -> SBUF bf16 ----
                    H2T = h2t_pool.tile([64, SUP * 128], BF16)
                    nc.scalar.activation(
                        H2T[:], psHn[:],
                        mybir.ActivationFunctionType.Silu,
                        bias=beta_sb[:], scale=1.0)
                    # ---- MM2 per tile ----
                    QT = psQT.tile([128, SUP * 96], F32)
                    for t in range(SUP):
                        nc.tensor.matmul(
                            QT[:, t * 96:(t + 1) * 96],
                            H2T[:, t * 128:(t + 1) * 128],
                            b_sb[:],
                            start=True, stop=True)

                    a3 = (auxg[:, si * SUP * 5:(si + 1) * SUP * 5]
                          .rearrange("p (t f) -> p t f", f=5))
                    deg4 = deg_bufs[(nt0 // SUP) % NDEG]
                    d3 = deg4[:].rearrange("p (t f) -> p t f", f=DEG_W)
                    q3 = QT[:].rearrange("p (t f) -> p t f", f=96)

                    bi = nt0 // SUP
                    # deg0 = a0 * q0  (batched; engine per config)
                    deg0_eng = CONFIG["deg0_engine"]
                    if deg0_eng == "mix":
                        deg0_eng = ("pool" if bi
                                    % CONFIG["deg0_pool_every"] == 0 else "dve")
                    d0_eng = nc.gpsimd if deg0_eng == "pool" else nc.vector
                    a0ex = a3[:, :, 0:1].broadcast_to([128, SUP, 64])
                    d0_eng.scalar_tensor_tensor(
                        d3[:, :, 0:64], q3[:, :, 0:64], 0.0, a0ex,
                        mybir.AluOpType.bypass, mybir.AluOpType.mult)

                    # deg1[m-major] = q1 x a1  (engine per config)
                    deg1_eng = CONFIG["deg1_engine"]
                    if deg1_eng == "mix":
                        deg1_eng = ("dve" if bi
                                    % CONFIG["deg1_dve_every"] == 0 else "pool")
                    d1_eng = nc.gpsimd if deg1_eng == "pool" else nc.vector
                    for m_ in range(3):
                        a1ex = (a3[:, :, 1 + m_:2 + m_]
                                .broadcast_to([128, SUP, 32]))
                        d1_eng.scalar_tensor_tensor(
                            d3[:, :, 64 + 32 * m_:96 + 32 * m_],
                            q3[:, :, 64:96], 0.0, a1ex,
                            mybir.AluOpType.bypass, mybir.AluOpType.mult)

                    # ---- onehot + scatter per tile ----
                    for t in range(SUP):
                        nt = nt0 + t
                        w = winof(nt)
                        tin = nt - w * TPW
                        oh_eng_name = CONFIG["onehot_engine"]
                        if oh_eng_name == "mix":
                            oh_eng_name = ("pool" if nt
                                           % CONFIG["onehot_pool_every"] == 0
                                           else "dve")
                        oh_eng = nc.gpsimd if oh_eng_name == "pool" else nc.vector
                        oh = oh_pool.tile([128, WIN], BF16)
                        oh_eng.tensor_scalar(oh[:], iota_sb[:],
                                             a3[:, t, 4:5], None,
                                             mybir.AluOpType.is_equal)
                        if tin == 0:
                            if acc is not None:
                                fl = fl_pool.tile([128, D_EMB], F32)
                                nc.vector.tensor_copy(fl[:], acc[:, 0:D_EMB])
                                nc.sync.dma_start(
                                    out_d[acc_win * 128:(acc_win + 1) * 128, :],
                                    fl[:])
                            acc = psA.tile([128, DEG_W], F32)
                            acc_win = w
                        is_last = (nt == NT - 1) or (winof(nt + 1) != w)
                        nc.tensor.matmul(
                            acc[:],
                            oh[:],
                            deg4[:, t * DEG_W:(t + 1) * DEG_W],
                            start=(tin == 0), stop=is_last,
                            skip_group_check=True)

            fl = fl_pool.tile([128, D_EMB], F32)
            nc.vector.tensor_copy(fl[:], acc[:, 0:D_EMB])
            nc.sync.dma_start(out_d[acc_win * 128:(acc_win + 1) * 128, :], fl[:])

    nc.finalize()
    return nc


def kernel(dst_input, src_attr, scalars, lin_w, lin_b, rad_w1, rad_g, rad_beta,
           rad_w2, rad_off, proj_w0, proj_b0, proj_w1, dst_index):
    dst_input = np.asarray(dst_input)
    src_attr = np.asarray(src_attr, np.float32)
    scalars = np.asarray(scalars, np.float32)
    lin_w = np.asarray(lin_w, np.float64)
    lin_b = np.asarray(lin_b, np.float64)
    rad_w1 = np.asarray(rad_w1, np.float32)
    rad_g = np.asarray(rad_g, np.float32)
    rad_beta = np.asarray(rad_beta, np.float32)
    rad_w2 = np.asarray(rad_w2, np.float64)
    rad_off = np.asarray(rad_off, np.float64)
    proj_w0 = np.asarray(proj_w0, np.float64)
    proj_b0 = np.asarray(proj_b0, np.float64)
    proj_w1 = np.asarray(proj_w1, np.float64)
    dst_index = np.asarray(dst_index)

    N = dst_input.shape[0]
    E = scalars.shape[0]
    out_dtype = dst_input.dtype

    # ---- host weight folds ----
    s0 = lin_w + lin_b                                   # [64]
    k0 = 1.0 / (math.sqrt(MUL0 + MUL1) * math.sqrt(AVG_AGG))
    k1 = 1.0 / (math.sqrt(MUL0 + 2 * MUL1) * math.sqrt(AVG_AGG))
    A0 = s0[:, None] * proj_w0[:MUL0, :]                 # [64, 64]
    A1 = s0[:, None] * proj_w1[:MUL0, :]                 # [64, 32]
    B0 = rad_w2[:, 0:64] @ A0 * k0                       # [64, 64]
    B1 = rad_w2[:, 64:128] @ A1 * k1                     # [64, 32]
    Bm = np.concatenate([B0, B1], axis=1)                # [64, 96] (m-major out)
    c0 = rad_off[0:64] @ A0 * k0                         # [64]
    c1 = rad_off[64:128] @ A1 * k1                       # [32]
    W1c = rad_w1 - rad_w1.mean(axis=1, keepdims=True)    # fold LN mean
    W1g = W1c * rad_g[None, :]                           # fold LN gain

    # ---- host rstd fold: x' = x * rstd(x) ----
    hc = scalars @ W1c                                   # [E, 64] f32
    var = np.mean(hc * hc, axis=1) + LN_EPS
    rstd = (1.0 / np.sqrt(var)).astype(np.float32)
    xprime = scalars * rstd[:, None]                     # [E, 64]

    # ---- node -> window bin packing (balance edges per 128-node window) ----
    import heapq
    NPC = (N + N_CORES - 1) // N_CORES
    NW = (NPC + WIN - 1) // WIN                          # windows per core
    NWIN = NW * N_CORES                                  # global window count
    deg = np.bincount(dst_index, minlength=N)
    node_order = np.argsort(-deg, kind="stable")
    heap = [(0, 0, w) for w in range(NWIN)]              # (load, count, win)
    heapq.heapify(heap)
    win_nodes = [[] for _ in range(NWIN)]
    win_load = np.zeros(NWIN, np.int64)
    for nd in node_order:
        while True:
            load, cntn, w = heapq.heappop(heap)
            if cntn < WIN:
                break                                    # full windows drop out
        win_nodes[w].append(int(nd))
        win_load[w] = load + deg[nd]
        heapq.heappush(heap, (int(win_load[w]), cntn + 1, w))
    # windows -> cores: sort by load desc, greedily to least-loaded core
    worder = np.argsort(-win_load, kind="stable")
    core_wins = [[] for _ in range(N_CORES)]
    core_load = np.zeros(N_CORES, np.int64)
    for w in worder:
        k = min((kk for kk in range(N_CORES) if len(core_wins[kk]) < NW),
                key=lambda kk: core_load[kk])
        core_wins[k].append(int(w))
        core_load[k] += win_load[w]

    TPW = max(1, int(np.ceil(win_load.max() / 128)))
    NT = NW * TPW
    NT = ((NT + SGT - 1) // SGT) * SGT                   # pad to DMA chunk
    C = NT * 128

    # per-node edge ranges in the dst-sorted order
    order = np.argsort(dst_index, kind="stable")
    starts = np.zeros(N + 1, np.int64)
    np.cumsum(deg, out=starts[1:])

    key = (C, NW, TPW, NT, tuple(sorted(CONFIG.items())))
    if key not in _PROGRAM_CACHE:
        _PROGRAM_CACHE[key] = build_program(C, NW, TPW, NT)
    nc = _PROGRAM_CACHE[key]

    # ---- per-core input arrays ----
    iota = np.broadcast_to(np.arange(WIN, dtype=np.float32)[None, :],
                           (128, WIN)).astype(BF16_NP)
    w1_b = np.ascontiguousarray(W1g.astype(np.float32)).astype(BF16_NP)
    b_b = np.ascontiguousarray(Bm.astype(np.float32)).astype(BF16_NP)
    beta_c = np.ascontiguousarray(rad_beta.reshape(64, 1).astype(np.float32))

    node_core = np.zeros(N, np.int32)
    node_row = np.zeros(N, np.int64)
    in_maps = []
    for k in range(N_CORES):
        xt = np.zeros((C, 64), np.float32)
        aux = np.zeros((NT, 128, 5), np.float32)
        aux[:, :, 4] = -1.0
        a = aux.reshape(NT * 128, 5)
        for wi, w in enumerate(core_wins[k]):
            nds = np.asarray(win_nodes[w], np.int64)
            if nds.size == 0:
                continue
            node_core[nds] = k
            node_row[nds] = wi * 128 + np.arange(nds.size)
            eidx = np.concatenate(
                [order[starts[nd]:starts[nd + 1]] for nd in nds])
            cnt = eidx.size
            base = wi * TPW * 128
            xt[base:base + cnt] = xprime[eidx]
            a[base:base + cnt, 0] = src_attr[eidx, 0]
            a[base:base + cnt, 1:4] = src_attr[eidx, 1:4]
            a[base:base + cnt, 4] = np.repeat(
                np.arange(nds.size, dtype=np.float32), deg[nds])
        auxp = np.ascontiguousarray(
            aux.reshape(NT // SGT, SGT, 128, 5).transpose(0, 2, 1, 3)
            .reshape(NT // SGT, 128, SGT * 5))
        m = {
            "xt": np.ascontiguousarray(xt.T).astype(BF16_NP),
            "aux": auxp,
            "w1g": w1_b,
            "bmat": b_b,
            "betac": beta_c,
            "iota": iota,
        }
        in_maps.append(m)

    global _LAST_IN_MAPS
    _LAST_IN_MAPS = in_maps
    res = run_bass_kernel_spmd(nc, in_maps, core_ids=list(range(N_CORES)))

    # ---- host assembly ----
    out = np.zeros((N, D_EMB), np.float64)
    for k in range(N_CORES):
        rows = np.asarray(res.results[k]["out"], np.float64)  # [NW*128, 160]
        mask = node_core == k
        out[mask] = rows[node_row[mask]]
    # device o1 layout is m-major (64 + 32*m + v); reference is 64 + 3*v + m
    blk = out[:, 64:160].reshape(N, 3, 32)
    out[:, 64:160] = blk.transpose(0, 2, 1).reshape(N, 96)

    # host-side exact corrections (rad_off and proj_b0 terms)
    if np.any(proj_b0 != 0) or np.any(c0 != 0) or np.any(c1 != 0):
        cnt = np.bincount(dst_index, minlength=N).astype(np.float64)
        suma0 = np.bincount(dst_index, weights=src_attr[:, 0].astype(np.float64),
                            minlength=N)
        out[:, 0:64] += cnt[:, None] * (proj_b0 / math.sqrt(AVG_AGG))[None, :]
        out[:, 0:64] += suma0[:, None] * c0[None, :]
        for m_ in range(3):
            sa = np.bincount(dst_index,
                             weights=src_attr[:, 1 + m_].astype(np.float64),
                             minlength=N)
            out[:, 64 + 3 * np.arange(32) + m_] += sa[:, None] * c1[None, :]

    return out.astype(out_dtype)
